# revision 1
# baseline (speedup 1.0000x reference)
"""Trainium2 Bass kernel for nn_Attention_54159537603130.

Dense GQA attention block (QKV proj + RoPE + causal attention + out proj),
sharded over 8 NeuronCores as (batch=2) x (kv-head groups=4).  Each core
computes a [S, DIM] partial of the output projection (wo is row-sharded);
the host sums the 4 group partials per batch.

All on-chip matmul operands live in "transposed" feature-on-partition
layouts so no large on-chip transposes are needed:
  Q^T/K^T [d, t]  -> scores^T tiles [t, s] directly
  V token-major [t, d] -> out^T = V^T @ P^T via PE accumulation
  out^T [d, s] is exactly the lhsT of the wo matmul.
Softmax runs without max-subtraction (logits are O(10) here); row sums are
computed with a ones-vector matmul, and normalization is applied to out^T
via a K=1 broadcast matmul + one vector multiply.
"""

import os
import sys

sys.path.insert(0, "/opt/trn_rl_repo")

import numpy as np
import ml_dtypes

import concourse.bass as bass
import concourse.tile as tile
from concourse import mybir

BF16 = mybir.dt.bfloat16
F32 = mybir.dt.float32
NPBF16 = ml_dtypes.bfloat16

DIM, NH, NKV, HD = 4096, 32, 8, 128
B, S = 2, 2048
NCORES = 8
GQ = 8  # q heads per core
GKV = 2  # kv heads per core
MQ = GQ * HD  # 1024 q-proj cols per core
MKV = GKV * HD  # 256 kv-proj cols per core
SC = 1.0 / np.sqrt(HD)
NEG_INF = -1e9

LAST_EXEC_TIME_NS = None
LAST_RESULTS = None


def _install_ntff_hook():
    """antenv.axon_hooks is absent in this image; reconstruct the NTFF
    profiling hook via ctypes against libaxon_pjrt.so (only used when
    BASS_TRACE is set)."""
    import types
    import contextlib
    import ctypes

    if "antenv.axon_hooks" in sys.modules:
        return
    try:
        lib = ctypes.CDLL("/opt/axon/libaxon_pjrt.so")
        have = hasattr(lib, "axon_start_nrt_profile")
    except OSError:
        have = False

    if have:
        lib.axon_start_nrt_profile.argtypes = [
            ctypes.POINTER(ctypes.c_int64),
            ctypes.c_size_t,
        ]
        lib.axon_start_nrt_profile.restype = ctypes.c_int64
        lib.axon_stop_nrt_profile.argtypes = [ctypes.c_char_p]
        lib.axon_stop_nrt_profile.restype = ctypes.c_int64

        @contextlib.contextmanager
        def _hook(output_dir, device_ids):
            import jax

            jax.devices()
            if device_ids:
                ids = (ctypes.c_int64 * len(device_ids))(*device_ids)
                rc = lib.axon_start_nrt_profile(ids, len(device_ids))
            else:
                rc = lib.axon_start_nrt_profile(None, 0)
            if rc != 0:
                raise RuntimeError(f"axon_start_nrt_profile rc={rc}")
            try:
                yield
            finally:
                n = lib.axon_stop_nrt_profile(str(output_dir).encode())
                print(f"profile: {n} file(s) written to {output_dir}")

        hook = _hook
    else:
        hook = None

    mod = types.ModuleType("antenv.axon_hooks")
    mod.get_axon_ntff_profile_hook = lambda: hook
    mod.set_axon_ntff_profile_hook = lambda h: None
    sys.modules["antenv.axon_hooks"] = mod


def split_excess_waits(nc, max_waits=1):
    """walrus codegen supports very few sync waits per instruction while
    Tile's tail/release drains can carry several; hoist excess onto NOPs."""
    for fn in nc.m.functions:
        for blk in fn.blocks:
            insts = blk.instructions
            changed = False
            i = 0
            while i < len(insts):
                inst = insts[i]
                si = inst.sync_info
                if (
                    si is not None
                    and si.on_wait is not None
                    and len(si.on_wait) > max_waits
                ):
                    w = si.on_wait
                    k = 0
                    while len(w) > max_waits:
                        nop = mybir.InstNoOp(
                            name=f"{inst.name}_wsplit{k}",
                            engine=inst.engine,
                            ins=[],
                            outs=[],
                        )
                        nop.sync_info = mybir.SyncInfo(
                            on_wait=w[:max_waits], on_update=[]
                        )
                        insts.insert(i, nop)
                        i += 1
                        w = w[max_waits:]
                        k += 1
                    inst.sync_info = mybir.SyncInfo(on_wait=w, on_update=si.on_update)
                    changed = True
                i += 1
            if changed:
                blk.instructions = insts


def _build(causal: bool):
    nc = bass.Bass("TRN2", target_bir_lowering=False, debug=False)
    Exp = mybir.ActivationFunctionType.Exp

    # DRAM I/O — all inputs pre-tiled on the host into SBUF-friendly
    # [partition, ...] layouts with large contiguous per-partition runs.
    xq_d = nc.dram_tensor("xq", [4, 128, 32, 512], BF16, kind="ExternalInput").ap()
    wq_d = nc.dram_tensor("wq", [8, 128, 32, 128], BF16, kind="ExternalInput").ap()
    wk_d = nc.dram_tensor("wk", [2, 128, 32, 128], BF16, kind="ExternalInput").ap()
    wv_d = nc.dram_tensor("wv", [128, 32, 256], BF16, kind="ExternalInput").ap()
    wo_d = nc.dram_tensor("wo", [128, 8, 4096], BF16, kind="ExternalInput").ap()
    ceq_d = nc.dram_tensor("ceq", [128, 2048], BF16, kind="ExternalInput").ap()
    s2q_d = nc.dram_tensor("s2q", [128, 2048], BF16, kind="ExternalInput").ap()
    cek_d = nc.dram_tensor("cek", [128, 2048], BF16, kind="ExternalInput").ap()
    s2k_d = nc.dram_tensor("s2k", [128, 2048], BF16, kind="ExternalInput").ap()
    psw_d = nc.dram_tensor("pswap", [128, 128], BF16, kind="ExternalInput").ap()
    if causal:
        mask_d = nc.dram_tensor(
            "maskd", [128, 16, 512], BF16, kind="ExternalInput"
        ).ap()
    else:
        mask_d = nc.dram_tensor(
            "maskt", [128, 16, 2048], BF16, kind="ExternalInput"
        ).ap()
    out_d = nc.dram_tensor("out", [128, 16, 4096], F32, kind="ExternalOutput").ap()

    with tile.TileContext(nc) as tc:
        with (
            tc.tile_pool(name="consts", bufs=1) as consts,
            tc.tile_pool(name="persist", bufs=1) as persist,
        ):
            ceq_t = consts.tile([128, 2048], BF16)
            nc.gpsimd.dma_start(out=ceq_t, in_=ceq_d)
            s2q_t = consts.tile([128, 2048], BF16)
            nc.gpsimd.dma_start(out=s2q_t, in_=s2q_d)
            cek_t = consts.tile([128, 2048], BF16)
            nc.gpsimd.dma_start(out=cek_t, in_=cek_d)
            s2k_t = consts.tile([128, 2048], BF16)
            nc.gpsimd.dma_start(out=s2k_t, in_=s2k_d)
            psw_t = consts.tile([128, 128], BF16)
            nc.gpsimd.dma_start(out=psw_t, in_=psw_d)
            ones_col = consts.tile([128, 1], BF16)
            nc.vector.memset(ones_col, 1.0)
            ones_row = consts.tile([1, 128], F32)
            nc.vector.memset(ones_row, 1.0)

            qrot = persist.tile([128, GQ, 2048], BF16)
            krot = persist.tile([128, GKV, 2048], BF16)
            vtok = persist.tile([128, 16, MKV], BF16)
            if causal:
                # small diagonal-band mask: prefetch before phase 1
                mask_t = persist.tile([128, 16, 512], BF16)
                nc.gpsimd.dma_start(out=mask_t, in_=mask_d)

            # ---------------- Phase 1: QKV projections + RoPE --------------
            with (
                tc.tile_pool(name="p1", bufs=1) as p1,
                tc.tile_pool(name="p1ps", bufs=1, space="PSUM") as pps,
            ):
                wv_t = p1.tile([128, 32, 256], BF16, tag="wv", bufs=1)
                nc.gpsimd.dma_start(out=wv_t, in_=wv_d)

                def rope(ps, ce, s2, dst, toff):
                    # dst = ce*q + s2*pairswap(q), all [128, 512] at t-offset toff
                    qb = p1.tile([128, 512], BF16, tag="ropeb", bufs=3)
                    nc.scalar.copy(out=qb, in_=ps)
                    sw = pps.tile([128, 512], F32, tag="swap", bufs=2)
                    nc.tensor.matmul(out=sw, lhsT=psw_t, rhs=qb, start=True, stop=True)
                    a = p1.tile([128, 512], BF16, tag="ropea", bufs=3)
                    nc.vector.tensor_mul(a, qb, ce[:, toff : toff + 512])
                    bt = p1.tile([128, 512], BF16, tag="ropec", bufs=3)
                    nc.vector.tensor_mul(bt, sw, s2[:, toff : toff + 512])
                    nc.vector.tensor_add(dst, a, bt)

                for q in range(4):
                    t0 = 512 * q
                    xh = p1.tile([128, 32, 512], BF16, tag="xh", bufs=2)
                    nc.gpsimd.dma_start(out=xh, in_=xq_d[q])
                    for m in range(GQ):
                        wqc = p1.tile([128, 32, 128], BF16, tag="wc", bufs=3)
                        nc.gpsimd.dma_start(out=wqc, in_=wq_d[m])
                        ps = pps.tile([128, 512], F32, tag="proj", bufs=2)
                        for d in range(32):
                            nc.tensor.matmul(
                                out=ps,
                                lhsT=wqc[:, d],
                                rhs=xh[:, d],
                                start=(d == 0),
                                stop=(d == 31),
                            )
                        rope(ps, ceq_t, s2q_t, qrot[:, m, t0 : t0 + 512], t0)
                    for m in range(GKV):
                        wkc = p1.tile([128, 32, 128], BF16, tag="wc", bufs=3)
                        nc.gpsimd.dma_start(out=wkc, in_=wk_d[m])
                        ps = pps.tile([128, 512], F32, tag="proj", bufs=2)
                        for d in range(32):
                            nc.tensor.matmul(
                                out=ps,
                                lhsT=wkc[:, d],
                                rhs=xh[:, d],
                                start=(d == 0),
                                stop=(d == 31),
                            )
                        rope(ps, cek_t, s2k_t, krot[:, m, t0 : t0 + 512], t0)
                    for tv in range(4):
                        psv = pps.tile([128, 256], F32, tag="vproj", bufs=2)
                        for d in range(32):
                            nc.tensor.matmul(
                                out=psv,
                                lhsT=xh[:, d, 128 * tv : 128 * tv + 128],
                                rhs=wv_t[:, d],
                                start=(d == 0),
                                stop=(d == 31),
                            )
                        nc.scalar.copy(out=vtok[:, 4 * q + tv, :], in_=psv)

            # outT lives from phase 2 through phase 3; allocated after the
            # phase-1 pools release so SBUF peaks stay under budget. wo is
            # prefetched here so its 8MB load overlaps phase 2.
            outT_pool = tc.alloc_tile_pool(name="po", bufs=1)
            outT = outT_pool.tile([128, GQ, 2048], BF16)
            wo_pool = tc.alloc_tile_pool(name="pwo", bufs=1)
            wo_t = wo_pool.tile([128, 8, 4096], BF16)
            nc.gpsimd.dma_start(out=wo_t, in_=wo_d)

            # ---------------- Phase 2: attention ---------------------------
            with (
                tc.tile_pool(name="p2", bufs=1) as p2,
                tc.tile_pool(name="p2ps", bufs=1, space="PSUM") as pps2,
            ):
                if not causal:
                    mask_t = p2.tile([128, 16, 2048], BF16)
                    nc.gpsimd.dma_start(out=mask_t, in_=mask_d)

                def finalize(fin):
                    # softmax denominator: 1/rowsum = exp(-ln(r)) on ScalarE
                    # (DVE reciprocal on a 1-partition row costs 3.3us and
                    # stalls the DVE queue), broadcast via K=1 matmul, then
                    # scale out^T. Emitted one block late so the PE never
                    # waits on this chain.
                    pso_, psr_, h_, s0_ = fin
                    # 1/r = exp(-ln r) on ScalarE (DVE reciprocal is 3.3us)
                    nc.scalar.activation(
                        out=psr_,
                        in_=psr_,
                        func=mybir.ActivationFunctionType.Ln,
                    )
                    rp = p2.tile([1, 512], F32, tag="rp", bufs=2)
                    nc.scalar.activation(
                        out=rp,
                        in_=psr_,
                        func=mybir.ActivationFunctionType.Exp,
                        scale=-1.0,
                    )
                    psb = pps2.tile([128, 512], F32, tag="psb", bufs=1)
                    nc.tensor.matmul(
                        out=psb, lhsT=ones_row, rhs=rp, start=True, stop=True
                    )
                    rb = p2.tile([128, 512], F32, tag="rb", bufs=2)
                    nc.vector.tensor_copy(out=rb, in_=psb)
                    nc.vector.tensor_mul(outT[:, h_, s0_ : s0_ + 512], pso_, rb)

                pending = None
                for h in range(GQ):
                    kv = h // 4
                    for j in range(4):
                        s0 = 512 * j
                        ilist = list(range(4 * (j + 1))) if causal else list(range(16))
                        n_i = len(ilist)
                        pso = pps2.tile([128, 512], F32, tag="pso", bufs=2)
                        psr = pps2.tile([1, 512], F32, tag="psr", bufs=2)
                        for idx, i in enumerate(ilist):
                            pss = pps2.tile([128, 512], F32, tag="pss", bufs=3)
                            nc.tensor.matmul(
                                out=pss,
                                lhsT=krot[:, kv, 128 * i : 128 * i + 128],
                                rhs=qrot[:, h, s0 : s0 + 512],
                                start=True,
                                stop=True,
                            )
                            if causal:
                                if i >= 4 * j:
                                    nc.vector.tensor_add(pss, pss, mask_t[:, i, :])
                            else:
                                nc.vector.tensor_add(
                                    pss, pss, mask_t[:, i, s0 : s0 + 512]
                                )
                            e = p2.tile([128, 512], BF16, tag="exp", bufs=6)
                            nc.scalar.activation(out=e, in_=pss, func=Exp)
                            nc.tensor.matmul(
                                out=pso,
                                lhsT=vtok[:, i, 128 * kv : 128 * kv + 128],
                                rhs=e,
                                start=(idx == 0),
                                stop=(idx == n_i - 1),
                            )
                            nc.tensor.matmul(
                                out=psr[0:1, :],
                                lhsT=ones_col,
                                rhs=e,
                                start=(idx == 0),
                                stop=(idx == n_i - 1),
                            )
                            if idx == 0 and pending is not None:
                                # finalize the previous block right after this
                                # block's first tile: its ACT ln/exp lands
                                # ahead of most of this block's exps, freeing
                                # PSUM slots early
                                finalize(pending)
                                pending = None
                        if pending is not None:
                            finalize(pending)
                        pending = (pso, psr, h, s0)
                finalize(pending)

            # ---------------- Phase 3: output projection -------------------
            with (
                tc.tile_pool(name="p3", bufs=1) as p3,
                tc.tile_pool(name="p3ps", bufs=1, space="PSUM") as pps3,
            ):
                # m-outer ordering: one LDWEIGHTS per 8 matmuls (all 8 PSUM
                # banks accumulate in parallel across the dc dimension)
                for s in range(16):
                    psfs = [
                        pps3.tile(
                            [128, 512], F32, tag="psf", bufs=8, name=f"psf_{s}_{dc}"
                        )
                        for dc in range(8)
                    ]
                    for m in range(8):
                        for dc in range(8):
                            nc.tensor.matmul(
                                out=psfs[dc],
                                lhsT=outT[:, m, 128 * s : 128 * s + 128],
                                rhs=wo_t[:, m, 512 * dc : 512 * dc + 512],
                                start=(m == 0),
                                stop=(m == 7),
                            )
                    for dc in range(8):
                        ot = p3.tile([128, 512], F32, tag="ot", bufs=8)
                        nc.scalar.copy(out=ot, in_=psfs[dc])
                        nc.gpsimd.dma_start(
                            out=out_d[:, s, 512 * dc : 512 * dc + 512], in_=ot
                        )

            wo_pool.release()
            outT_pool.release()

    return nc


def _prep_inputs(x, freqs_cos, freqs_sin, mask, wq, wk, wv, wo, causal):
    """Host-side shard + retile into the DMA layouts declared in _build."""
    f32 = np.float32

    # RoPE planes [128, 2048]: ce[2i,t]=ce[2i+1,t]=cos[t,i];
    # s2[2i,t]=-sin[t,i], s2[2i+1,t]=+sin[t,i]. Query planes carry 1/sqrt(HD).
    cos_t = np.asarray(freqs_cos, f32).T  # [64, 2048]
    sin_t = np.asarray(freqs_sin, f32).T
    ce = np.repeat(cos_t, 2, axis=0)  # [128, 2048]
    s2 = np.empty((HD, S), f32)
    s2[0::2] = -sin_t
    s2[1::2] = sin_t
    ceq = (ce * SC).astype(NPBF16)
    s2q = (s2 * SC).astype(NPBF16)
    cek = ce.astype(NPBF16)
    s2k = s2.astype(NPBF16)

    # pair-swap permutation
    psw = np.zeros((HD, HD), NPBF16)
    idx = np.arange(HD)
    psw[idx ^ 1, idx] = 1

    maskT = np.ascontiguousarray(np.asarray(mask, f32).T)  # [t, s]
    if causal:
        band = np.empty((16, 128, 512), f32)
        for i in range(16):
            j = i // 4
            band[i] = maskT[128 * i : 128 * (i + 1), 512 * j : 512 * j + 512]
        mask_host = np.ascontiguousarray(band.transpose(1, 0, 2)).astype(NPBF16)
    else:
        mask_host = np.ascontiguousarray(
            maskT.reshape(16, 128, 2048).transpose(1, 0, 2)
        ).astype(NPBF16)

    # per-batch x tiles [4, 128, 32, 512]
    xq_b = []
    for b in range(B):
        xT = np.asarray(x[b], f32).astype(NPBF16).T  # [4096, 2048]
        xq_b.append(
            np.ascontiguousarray(
                xT.reshape(32, 128, 4, 512).transpose(2, 1, 0, 3)
            )
        )

    # per-group weight tiles
    wq_g, wk_g, wv_g, wo_g = [], [], [], []
    for g in range(4):
        wqs = np.asarray(wq[:, MQ * g : MQ * (g + 1)], f32).astype(NPBF16)
        wq_g.append(
            np.ascontiguousarray(wqs.reshape(32, 128, 8, 128).transpose(2, 1, 0, 3))
        )
        wks = np.asarray(wk[:, MKV * g : MKV * (g + 1)], f32).astype(NPBF16)
        wk_g.append(
            np.ascontiguousarray(wks.reshape(32, 128, 2, 128).transpose(2, 1, 0, 3))
        )
        wvs = np.asarray(wv[:, MKV * g : MKV * (g + 1)], f32).astype(NPBF16)
        wv_g.append(np.ascontiguousarray(wvs.reshape(32, 128, 256).transpose(1, 0, 2)))
        wos = np.asarray(wo[MQ * g : MQ * (g + 1), :], f32).astype(NPBF16)
        wo_g.append(np.ascontiguousarray(wos.reshape(8, 128, 4096).transpose(1, 0, 2)))

    mask_key = "maskd" if causal else "maskt"
    in_maps = []
    for c in range(NCORES):
        b, g = c // 4, c % 4
        in_maps.append(
            {
                "xq": xq_b[b],
                "wq": wq_g[g],
                "wk": wk_g[g],
                "wv": wv_g[g],
                "wo": wo_g[g],
                "ceq": ceq,
                "s2q": s2q,
                "cek": cek,
                "s2k": s2k,
                "pswap": psw,
                mask_key: mask_host,
            }
        )
    return in_maps


def kernel(x, start_pos, freqs_cos, freqs_sin, cache, mask, wq, wk, wv, wo):
    global LAST_EXEC_TIME_NS, LAST_RESULTS

    x = np.asarray(x)
    mask = np.asarray(mask)
    assert x.shape == (B, S, DIM), x.shape
    assert int(start_pos) == 0, "kernel specialized for start_pos=0"

    causal_ref = np.where(
        np.arange(S)[None, :] <= np.arange(S)[:, None], 0.0, NEG_INF
    ).astype(np.float32)
    causal = bool(np.array_equal(np.asarray(mask, np.float32), causal_ref))

    _install_ntff_hook()
    from concourse.bass_utils import run_bass_kernel_spmd
    import concourse.bass_utils as _bu

    trace = bool(os.environ.get("BASS_TRACE"))
    if trace:
        _bu.upload_artifacts = lambda tmpdir: tmpdir

    in_maps = _prep_inputs(x, freqs_cos, freqs_sin, mask, wq, wk, wv, wo, causal)
    nc = _build(causal)
    split_excess_waits(nc)

    res = run_bass_kernel_spmd(nc, in_maps, core_ids=list(range(NCORES)), trace=trace)
    LAST_EXEC_TIME_NS = res.exec_time_ns
    LAST_RESULTS = res

    partials = []
    for c in range(NCORES):
        o = res.results[c]["out"]  # [128, 16, 4096] f32, p-major token tiles
        partials.append(o.transpose(1, 0, 2).reshape(S, DIM))
    out = np.stack(
        [
            partials[0] + partials[1] + partials[2] + partials[3],
            partials[4] + partials[5] + partials[6] + partials[7],
        ]
    ).astype(np.float32)
    return out

